# revision 21
# baseline (speedup 1.0000x reference)
"""Trainium2 Bass kernel for 4D valid convolution.

x (2,2,32,32,64,64) f32, weight (4,2,3,3,3,3) f32, bias (4,) f32
-> out (2,4,30,30,62,62) f32  (valid cross-correlation + bias)

Strategy: 8 cores = batch(2) x a-quadrant(4). Each core computes
out[b, :, a_sel, :, :, :] from slab x[b, :, a0:a0+10, :, :, :].

TensorE mapping per core (bf16 inputs, f32 PSUM accumulate):
  K (contraction, partitions) = (b-window=6, ci=2, a-window=10) = 120
  M (psum partitions)         = (co=4, a_out=8, b_out=4) = 128
  N (streamed free dim)       = contiguous (c,d) output pixels, <=496
Host prebuilds banded lhsT matrices (one per (k,l) tap, side by side in
one [120, 9*128] array -> a single DMA); the 9 (k,l) taps accumulate in
PSUM using (c,d)-shifted views of the same SBUF x tile.

Optimizations vs the 145us version (measured ~130us, ~1.12x):
 - Host restages x into xp[(b2,ci,a), (c,d)] = [640, 4096] so any
   b-window is a contiguous [120, 4096] 2D DMA.
 - bb=0's tile loads in 5 column pieces, alternating Sync/GpSimd
   queues, so the cc-chunk matmuls start after the first ~153KB
   instead of the full 983KB; later prefetches are queued strictly
   BEHIND these pieces on the same DMA rings (ring FIFO = priority)
   so they cannot steal bandwidth from the critical first tile.
 - 56 dummy matmuls on a zeroed SBUF tile (~5.5us coverage) run during
   the initial DMA wait so the PE_HAM clock gate is released and no
   core lets the PE idle long enough to re-throttle to 1.2GHz.
 - The last b-block (b_out 28:30, M=64) runs its 8 c-chunks as 4
   column-tiled PAIRS: two concurrent M=64 matmuls in PE column groups
   (tile_position (0,0) / (0,64)) halve that block's stream time; the
   final pair computes in two column phases so its first add+store
   overlap its second half's matmuls.
 - Tail stores spread across the Sync/GpSimd/ACT queues' DMA rings.
Resulting timeline per core: ~7us fixed NEFF preamble, ~5.5us warm-up
(hiding first-tile DMA), ~111us gapless matmul stream (the bf16
streaming roofline for this banded scheme: 128 outputs per 9 taps per
cycle is provably optimal with K<=128), ~2.6us store tail, ~3.5us
fixed semaphore drain.
Output goes to DRAM partition-major per (b-block, c-chunk); the host
unscrambles.
"""

import sys

if "/opt/trn_rl_repo" not in sys.path:
    sys.path.insert(0, "/opt/trn_rl_repo")

import ml_dtypes
import numpy as np

BF16 = ml_dtypes.bfloat16

B, CI, CO = 2, 2, 4
A, B2, C, D = 32, 32, 64, 64
AO, BO, CL, DL = 30, 30, 62, 62
K = 3

# per-core a-slab starts; each core computes 8 output a-rows (q=3 overlaps q=2)
A0 = [0, 8, 16, 22]
SA = 10  # a-window (8 outputs + 2 halo)
SB = 6  # b-window per block (4 outputs + 2 halo)
NBB = 8  # b_out blocks: 7 full (4 wide) + 1 last (2 wide)
NCC = 8  # c chunks: 7 full (8 wide) + 1 last (6 wide)
# Dummy matmuls to hold the PE busy until real data lands. Coverage is
# ~32*107ns (cold) + 24*75ns (after the HAM gate releases) ~= 5.2us,
# sized to the slowest core's first-tile arrival so no core lets the PE
# idle >1 HAM window (which would re-throttle the clock to 1.2GHz).
N_WARM = 56
# bb=0 column-piece boundaries (in c units); cc chunk k's reads fall in
# pieces covering c in [8k, 8k+10)
PIECES = [(0, 10), (10, 18), (18, 34), (34, 50), (50, 64)]

_CACHE = {}


def _build_weights(weight: np.ndarray, bias: np.ndarray):
    """Banded lhsT matrices per (k,l) tap, plus per-partition bias vectors."""
    w = weight.astype(np.float32)

    def banded(sa, n_ao, sb, n_bo):
        # sel[d, o, t] = 1 if d == o + t
        sa_sel = np.zeros((sa, n_ao, K), np.float32)
        for t in range(K):
            for o in range(n_ao):
                sa_sel[o + t, o, t] = 1.0
        sb_sel = np.zeros((sb, n_bo, K), np.float32)
        for t in range(K):
            for o in range(n_bo):
                sb_sel[o + t, o, t] = 1.0
        # lhsT[(db,ci,da), t=(k,l), (co,ao,bo)] — taps side by side in columns
        out = np.zeros((sb * CI * sa, 9, CO * n_ao * n_bo), np.float32)
        for k in range(K):
            for l in range(K):
                wkl = w[:, :, :, :, k, l]  # (co, ci, i, j)
                m = np.einsum("dai,ebj,ocij->ecdoab", sa_sel, sb_sel, wkl)
                out[:, k * 3 + l, :] = m.reshape(sb * CI * sa, CO * n_ao * n_bo)
        return np.ascontiguousarray(out.reshape(sb * CI * sa, 9 * CO * n_ao * n_bo))

    w_main = banded(SA, 8, SB, 4)  # (120, 9*128)
    w_last = banded(SA, 8, 4, 2)  # (80, 9*64)
    bias_main = np.repeat(bias.astype(np.float32), 32).reshape(128, 1)
    # bias for the column-tiled pair blocks: two stacked M=64 bias patterns
    bias_pair = np.concatenate(
        [np.repeat(bias.astype(np.float32), 16)] * 2
    ).reshape(128, 1)
    return w_main, w_last, bias_main, bias_pair


def _build_program():
    import concourse.bass as bass  # noqa: F401
    import concourse.mybir as mybir
    import concourse.tile as tile
    from concourse import bacc

    f32 = mybir.dt.float32
    bf16 = mybir.dt.bfloat16

    nc = bacc.Bacc("TRN2", target_bir_lowering=False, debug=False, num_devices=8)
    # xp rows are (b2, ci, a) so any b-window of 6 rows is a contiguous
    # [120, 4096] block -> single clean 2D DMA per b-block.
    xs = nc.dram_tensor("xp", [B2 * CI * SA, C * D], bf16, kind="ExternalInput")
    wm = nc.dram_tensor("w_main", [120, 9 * 128], bf16, kind="ExternalInput")
    wl = nc.dram_tensor("w_last", [80, 9 * 64], bf16, kind="ExternalInput")
    bm = nc.dram_tensor("bias_main", [128, 1], f32, kind="ExternalInput")
    bp = nc.dram_tensor("bias_pair", [128, 1], f32, kind="ExternalInput")
    # partition-major blocks: [bb, cc, m, n]; host unscrambles (cheap numpy)
    out = nc.dram_tensor(
        "out_blocks", [NBB, NCC, 128, 8 * DL], f32, kind="ExternalOutput"
    )

    with tile.TileContext(nc) as tc:
        with (
            tc.tile_pool(name="w", bufs=1) as wpool,
            tc.tile_pool(name="rhs", bufs=2) as rpool,
            tc.tile_pool(name="psum", bufs=8, space="PSUM") as ppool,
            tc.tile_pool(name="ot", bufs=6) as opool,
        ):
            # --- PE pre-warm: emitted first so it heads the PE queue.
            # the warm psum comes from the main ring (it is consumed long
            # before the ring wraps), so all 8 PSUM banks serve the body
            # memset on the (otherwise idle) DVE queue: gpsimd carries the
            # pool-alloc instructions and early DMA issues, which the memset
            # would delay
            warm_src = wpool.tile([128, 128], bf16)
            nc.vector.memset(warm_src[:], 0.0)
            warm_ps = ppool.tile([128, 8 * DL], f32, tag="ps")
            for _ in range(N_WARM):
                nc.tensor.matmul(
                    warm_ps[:, :128],
                    warm_src[:],
                    warm_src[:],
                    start=True,
                    stop=True,
                )

            # --- weights + biases on the ACT queue (own DMA rings; don't
            # compete with the Sync/GpSimd rings carrying x data).
            w_main_t = wpool.tile([120, 9 * 128], bf16)
            nc.scalar.dma_start(w_main_t[:], wm[:])
            bias_main_t = wpool.tile([128, 1], f32)
            w_last_t = wpool.tile([80, 9 * 64], bf16)
            bias_pair_t = wpool.tile([128, 1], f32)
            nc.scalar.dma_start(bias_main_t[:], bm[:])
            nc.scalar.dma_start(w_last_t[:], wl[:])
            nc.scalar.dma_start(bias_pair_t[:], bp[:])

            # Queue plan (per-ring FIFO ordering = transfer priority):
            #   sync  : bb0-p0, bb0-p2, bb4, bb6
            #   gpsimd: bb0-p1, bb0-p3, bb1, bb2, bb3, bb5, bb7
            rhs_tiles = []
            for bb in range(NBB):
                wb = SB if bb < NBB - 1 else 4
                P = CI * SA * wb  # 120 or 80
                row0 = bb * 4 * CI * SA
                rhs_t = rpool.tile([P, C * D], bf16, tag="rhs")
                rhs_tiles.append(rhs_t)
                if bb == 0:
                    for i, (c0, c1) in enumerate(PIECES):
                        q = nc.sync if i % 2 == 0 else nc.gpsimd
                        q.dma_start(
                            rhs_t[:, c0 * D : c1 * D],
                            xs[row0 : row0 + P, c0 * D : c1 * D],
                        )
                elif bb in (1, 2, 3, 5, 7):
                    nc.gpsimd.dma_start(rhs_t[:], xs[row0 : row0 + P, :])
                else:
                    nc.sync.dma_start(rhs_t[:], xs[row0 : row0 + P, :])

            for bb in range(NBB - 1):  # full blocks: M=128
                rhs3 = rhs_tiles[bb].rearrange("p (c d) -> p c d", c=C)
                for cc in range(NCC):
                    c0 = cc * 8
                    wc = 8 if cc < NCC - 1 else 6
                    N = wc * DL
                    ps = ppool.tile([128, N], f32, tag="ps")
                    for t in range(9):
                        k, l = divmod(t, 3)
                        rv = rhs3[:, c0 + k : c0 + k + wc, l : l + DL]
                        nc.tensor.matmul(
                            ps.rearrange("m (c d) -> m c d", c=wc),
                            w_main_t[:, t * 128 : (t + 1) * 128],
                            rv,
                            start=(t == 0),
                            stop=(t == 8),
                        )
                    ot = opool.tile([128, N], f32, tag="ot")
                    nc.vector.tensor_scalar_add(ot[:], ps[:], bias_main_t[:])
                    nc.scalar.dma_start(out[bb, cc, :, :N], ot[:])

            # last block (b_out 28:30, M=64): run c-chunks as column-tiled
            # pairs — chunk 2p in PE columns 0:64, chunk 2p+1 in 64:128.
            # p=3 (the ragged pair with a split store) runs first so the
            # kernel's final store is a single clean block.
            rhs3 = rhs_tiles[NBB - 1].rearrange("p (c d) -> p c d", c=C)
            for p in (3, 0, 1, 2):
                wc_b = 8 if p < 3 else 6
                N_b = wc_b * DL
                ps = ppool.tile([128, 8 * DL], f32, tag="ps")
                ot = opool.tile([128, 8 * DL], f32, tag="ot")
                # the final pair (p=2) computes in two column phases so the
                # left phase's bias-add + store overlap the right phase's
                # matmuls, shortening the kernel's tail chain
                phases = [(0, 5), (5, 3)] if p == 2 else [(0, 8)]
                for pc0, pwc in phases:
                    for t in range(9):
                        k, l = divmod(t, 3)
                        for half, wch in enumerate([8, wc_b]):
                            w0 = min(pc0, wch)
                            w1 = min(pc0 + pwc, wch)
                            if w1 <= w0:
                                continue
                            c0 = 16 * p + 8 * half + w0
                            rv = rhs3[:, c0 + k : c0 + k + (w1 - w0), l : l + DL]
                            nc.tensor.matmul(
                                ps[
                                    64 * half : 64 * half + 64, w0 * DL : w1 * DL
                                ].rearrange("m (c d) -> m c d", c=w1 - w0),
                                w_last_t[:, t * 64 : (t + 1) * 64],
                                rv,
                                start=(t == 0),
                                stop=(t == 8),
                                tile_position=(0, 64 * half),
                            )
                    if p == 2:
                        lo, hi = pc0 * DL, (pc0 + pwc) * DL
                        nc.vector.tensor_scalar_add(
                            ot[:, lo:hi], ps[:, lo:hi], bias_pair_t[:]
                        )
                        if pc0 == 0:
                            nc.scalar.dma_start(
                                out[NBB - 1, 2 * p, :, lo:hi], ot[:, lo:hi]
                            )
                        else:
                            # final store: split by PARTITIONS (descriptors
                            # are per partition-row, so this halves the
                            # descriptor count per ring-set)
                            nc.sync.dma_start(
                                out[NBB - 1, 2 * p, :64, lo:hi], ot[:64, lo:hi]
                            )
                            nc.gpsimd.dma_start(
                                out[NBB - 1, 2 * p, 64:, lo:hi], ot[64:, lo:hi]
                            )
                if p == 1:
                    # second-to-last store on the (idle) gpsimd rings so it
                    # doesn't queue ahead of the final stores
                    nc.vector.tensor_scalar_add(ot[:], ps[:], bias_pair_t[:])
                    nc.gpsimd.dma_start(out[NBB - 1, 2 * p, :, :], ot[:])
                elif p == 0:
                    nc.vector.tensor_scalar_add(ot[:], ps[:], bias_pair_t[:])
                    nc.scalar.dma_start(out[NBB - 1, 2 * p, :, :], ot[:])
                elif p == 3:
                    nc.vector.tensor_scalar_add(
                        ot[:, :N_b], ps[:, :N_b], bias_pair_t[:]
                    )
                    nc.vector.tensor_scalar_add(
                        ot[:64, N_b : 8 * DL],
                        ps[:64, N_b : 8 * DL],
                        bias_pair_t[:64],
                    )
                    nc.scalar.dma_start(out[NBB - 1, 2 * p, :, :N_b], ot[:, :N_b])
                    nc.scalar.dma_start(
                        out[NBB - 1, 2 * p, :64, N_b : 8 * DL],
                        ot[:64, N_b : 8 * DL],
                    )
    nc.compile()
    return nc


def kernel(x: np.ndarray, weight: np.ndarray, bias: np.ndarray) -> np.ndarray:
    from concourse.bass_utils import run_bass_kernel_spmd

    if "nc" not in _CACHE:
        _CACHE["nc"] = _build_program()
    nc = _CACHE["nc"]

    w_main, w_last, bias_main, bias_pair = _build_weights(weight, bias)
    x_bf = x.astype(BF16)
    w_main = w_main.astype(BF16)
    w_last = w_last.astype(BF16)

    in_maps = []
    for core in range(8):
        b, q = divmod(core, 4)
        a0 = A0[q]
        slab = x_bf[b, :, a0 : a0 + SA]  # (ci, a, b2, c, d)
        xp = np.ascontiguousarray(slab.transpose(2, 0, 1, 3, 4)).reshape(
            B2 * CI * SA, C * D
        )
        in_maps.append(
            {
                "xp": xp,
                "w_main": w_main,
                "w_last": w_last,
                "bias_main": bias_main,
                "bias_pair": bias_pair,
            }
        )

    res = run_bass_kernel_spmd(nc, in_maps, core_ids=list(range(8)))
    _CACHE["last_result"] = res

    out = np.empty((B, CO, AO, BO, CL, DL), np.float32)
    for core in range(8):
        b, q = divmod(core, 4)
        slab = _unscramble(res.results[core]["out_blocks"])  # (4, 8, 30, 62, 62)
        if q < 3:
            out[b, :, 8 * q : 8 * q + 8] = slab
        else:
            out[b, :, 24:30] = slab[:, 2:8]
    return out


def _unscramble(blocks: np.ndarray) -> np.ndarray:
    """[NBB, NCC, 128, 8*62] partition-major blocks -> (4, 8, 30, 62, 62) slab."""
    slab = np.empty((CO, 8, BO, CL, DL), np.float32)
    for bb in range(NBB - 1):
        for cc in range(NCC):
            wc = 8 if cc < NCC - 1 else 6
            n = wc * DL
            blk = blocks[bb, cc, :, :n].reshape(CO, 8, 4, wc, DL)
            slab[:, :, bb * 4 : bb * 4 + 4, cc * 8 : cc * 8 + wc, :] = blk
    # last b-block: pairs stored at [7, 2p]; top half = chunk 2p (wc=8),
    # bottom half = chunk 2p+1 (wc=8, or 6 for p=3)
    for p in range(4):
        blk = blocks[NBB - 1, 2 * p]
        top = blk[:64, : 8 * DL].reshape(CO, 8, 2, 8, DL)
        slab[:, :, 28:30, 16 * p : 16 * p + 8, :] = top
        wc_b = 8 if p < 3 else 6
        bot = blk[64:128, : wc_b * DL].reshape(CO, 8, 2, wc_b, DL)
        slab[:, :, 28:30, 16 * p + 8 : 16 * p + 8 + wc_b, :] = bot
    return slab


# revision 23
# speedup vs baseline: 1.0000x; 1.0000x over previous
"""Trainium2 Bass kernel for 4D valid convolution.

x (2,2,32,32,64,64) f32, weight (4,2,3,3,3,3) f32, bias (4,) f32
-> out (2,4,30,30,62,62) f32  (valid cross-correlation + bias)

Strategy: 8 cores = batch(2) x a-quadrant(4). Each core computes
out[b, :, a_sel, :, :, :] from slab x[b, :, a0:a0+10, :, :, :].

TensorE mapping per core (bf16 inputs, f32 PSUM accumulate):
  K (contraction, partitions) = (b-window=6, ci=2, a-window=10) = 120
  M (psum partitions)         = (co=4, a_out=8, b_out=4) = 128
  N (streamed free dim)       = contiguous (c,d) output pixels, <=496
Host prebuilds banded lhsT matrices (one per (k,l) tap, side by side in
one [120, 9*128] array -> a single DMA); the 9 (k,l) taps accumulate in
PSUM using (c,d)-shifted views of the same SBUF x tile.

Optimizations vs the 145us version (measured ~130us, ~1.12x):
 - Host restages x into xp[(b2,ci,a), (c,d)] = [640, 4096] so any
   b-window is a contiguous [120, 4096] 2D DMA.
 - bb=0's tile loads in 5 column pieces, alternating Sync/GpSimd
   queues, so the cc-chunk matmuls start after the first ~153KB
   instead of the full 983KB; later prefetches are queued strictly
   BEHIND these pieces on the same DMA rings (ring FIFO = priority)
   so they cannot steal bandwidth from the critical first tile.
 - 56 dummy matmuls on a zeroed SBUF tile (~5.5us coverage) run during
   the initial DMA wait so the PE_HAM clock gate is released and no
   core lets the PE idle long enough to re-throttle to 1.2GHz.
 - The last b-block (b_out 28:30, M=64) runs its 8 c-chunks as 4
   column-tiled PAIRS: two concurrent M=64 matmuls in PE column groups
   (tile_position (0,0) / (0,64)) halve that block's stream time; the
   final pair computes in two column phases so its first add+store
   overlap its second half's matmuls.
 - Tail stores spread across the Sync/GpSimd/ACT queues' DMA rings.
Resulting timeline per core: ~7us fixed NEFF preamble, ~5.5us warm-up
(hiding first-tile DMA), ~111us gapless matmul stream (the bf16
streaming roofline for this banded scheme: 128 outputs per 9 taps per
cycle is provably optimal with K<=128), ~2.6us store tail, ~3.5us
fixed semaphore drain.
Output goes to DRAM partition-major per (b-block, c-chunk); the host
unscrambles.
"""

import sys

if "/opt/trn_rl_repo" not in sys.path:
    sys.path.insert(0, "/opt/trn_rl_repo")

import ml_dtypes
import numpy as np

BF16 = ml_dtypes.bfloat16

B, CI, CO = 2, 2, 4
A, B2, C, D = 32, 32, 64, 64
AO, BO, CL, DL = 30, 30, 62, 62
K = 3

# per-core a-slab starts; each core computes 8 output a-rows (q=3 overlaps q=2)
A0 = [0, 8, 16, 22]
SA = 10  # a-window (8 outputs + 2 halo)
SB = 6  # b-window per block (4 outputs + 2 halo)
NBB = 8  # b_out blocks: 7 full (4 wide) + 1 last (2 wide)
NCC = 8  # c chunks: 7 full (8 wide) + 1 last (6 wide)
# Dummy matmuls to hold the PE busy until real data lands. Coverage is
# ~32*107ns (cold) + 24*75ns (after the HAM gate releases) ~= 5.2us,
# sized to the slowest core's first-tile arrival so no core lets the PE
# idle >1 HAM window (which would re-throttle the clock to 1.2GHz).
N_WARM = 52
# bb=0 column-piece boundaries (in c units); cc chunk k's reads fall in
# pieces covering c in [8k, 8k+10)
PIECES = [(0, 10), (10, 18), (18, 34), (34, 50), (50, 64)]

_CACHE = {}


def _build_weights(weight: np.ndarray, bias: np.ndarray):
    """Banded lhsT matrices per (k,l) tap, plus per-partition bias vectors."""
    w = weight.astype(np.float32)

    def banded(sa, n_ao, sb, n_bo):
        # sel[d, o, t] = 1 if d == o + t
        sa_sel = np.zeros((sa, n_ao, K), np.float32)
        for t in range(K):
            for o in range(n_ao):
                sa_sel[o + t, o, t] = 1.0
        sb_sel = np.zeros((sb, n_bo, K), np.float32)
        for t in range(K):
            for o in range(n_bo):
                sb_sel[o + t, o, t] = 1.0
        # lhsT[(db,ci,da), t=(k,l), (co,ao,bo)] — taps side by side in columns
        out = np.zeros((sb * CI * sa, 9, CO * n_ao * n_bo), np.float32)
        for k in range(K):
            for l in range(K):
                wkl = w[:, :, :, :, k, l]  # (co, ci, i, j)
                m = np.einsum("dai,ebj,ocij->ecdoab", sa_sel, sb_sel, wkl)
                out[:, k * 3 + l, :] = m.reshape(sb * CI * sa, CO * n_ao * n_bo)
        return np.ascontiguousarray(out.reshape(sb * CI * sa, 9 * CO * n_ao * n_bo))

    w_main = banded(SA, 8, SB, 4)  # (120, 9*128)
    w_last = banded(SA, 8, 4, 2)  # (80, 9*64)
    bias_main = np.repeat(bias.astype(np.float32), 32).reshape(128, 1)
    # bias for the column-tiled pair blocks: two stacked M=64 bias patterns
    bias_pair = np.concatenate(
        [np.repeat(bias.astype(np.float32), 16)] * 2
    ).reshape(128, 1)
    return w_main, w_last, bias_main, bias_pair


def _build_program():
    import concourse.bass as bass  # noqa: F401
    import concourse.mybir as mybir
    import concourse.tile as tile
    from concourse import bacc

    f32 = mybir.dt.float32
    bf16 = mybir.dt.bfloat16

    nc = bacc.Bacc("TRN2", target_bir_lowering=False, debug=False, num_devices=8)
    # xp rows are (b2, ci, a) so any b-window of 6 rows is a contiguous
    # [120, 4096] block -> single clean 2D DMA per b-block.
    xs = nc.dram_tensor("xp", [B2 * CI * SA, C * D], bf16, kind="ExternalInput")
    wm = nc.dram_tensor("w_main", [120, 9 * 128], bf16, kind="ExternalInput")
    wl = nc.dram_tensor("w_last", [80, 9 * 64], bf16, kind="ExternalInput")
    bm = nc.dram_tensor("bias_main", [128, 1], f32, kind="ExternalInput")
    bp = nc.dram_tensor("bias_pair", [128, 1], f32, kind="ExternalInput")
    # partition-major blocks: [bb, cc, m, n]; host unscrambles (cheap numpy)
    out = nc.dram_tensor(
        "out_blocks", [NBB, NCC, 128, 8 * DL], f32, kind="ExternalOutput"
    )

    with tile.TileContext(nc) as tc:
        with (
            tc.tile_pool(name="w", bufs=1) as wpool,
            tc.tile_pool(name="rhs", bufs=2) as rpool,
            tc.tile_pool(name="psum", bufs=8, space="PSUM") as ppool,
            tc.tile_pool(name="ot", bufs=6) as opool,
        ):
            # --- PE pre-warm: emitted first so it heads the PE queue.
            # the warm psum comes from the main ring (it is consumed long
            # before the ring wraps), so all 8 PSUM banks serve the body
            warm_src = wpool.tile([128, 128], bf16)
            nc.gpsimd.memset(warm_src[:], 0.0)
            warm_ps = ppool.tile([128, 8 * DL], f32, tag="ps")
            for _ in range(N_WARM):
                nc.tensor.matmul(
                    warm_ps[:, :128],
                    warm_src[:],
                    warm_src[:],
                    start=True,
                    stop=True,
                )

            # --- weights + biases on the ACT queue (own DMA rings; don't
            # compete with the Sync/GpSimd rings carrying x data).
            w_main_t = wpool.tile([120, 9 * 128], bf16)
            nc.scalar.dma_start(w_main_t[:], wm[:])
            bias_main_t = wpool.tile([128, 1], f32)
            w_last_t = wpool.tile([80, 9 * 64], bf16)
            bias_pair_t = wpool.tile([128, 1], f32)
            nc.scalar.dma_start(bias_main_t[:], bm[:])
            nc.scalar.dma_start(w_last_t[:], wl[:])
            nc.scalar.dma_start(bias_pair_t[:], bp[:])

            # Queue plan (per-ring FIFO ordering = transfer priority):
            #   sync  : bb0-p0, bb0-p2, bb4, bb6
            #   gpsimd: bb0-p1, bb0-p3, bb1, bb2, bb3, bb5, bb7
            rhs_tiles = []
            for bb in range(NBB):
                wb = SB if bb < NBB - 1 else 4
                P = CI * SA * wb  # 120 or 80
                row0 = bb * 4 * CI * SA
                rhs_t = rpool.tile([P, C * D], bf16, tag="rhs")
                rhs_tiles.append(rhs_t)
                if bb == 0:
                    for i, (c0, c1) in enumerate(PIECES):
                        q = nc.sync if i % 2 == 0 else nc.gpsimd
                        q.dma_start(
                            rhs_t[:, c0 * D : c1 * D],
                            xs[row0 : row0 + P, c0 * D : c1 * D],
                        )
                elif bb in (1, 2, 3, 5, 7):
                    nc.gpsimd.dma_start(rhs_t[:], xs[row0 : row0 + P, :])
                else:
                    nc.sync.dma_start(rhs_t[:], xs[row0 : row0 + P, :])

            for bb in range(NBB - 1):  # full blocks: M=128
                rhs3 = rhs_tiles[bb].rearrange("p (c d) -> p c d", c=C)
                for cc in range(NCC):
                    c0 = cc * 8
                    wc = 8 if cc < NCC - 1 else 6
                    N = wc * DL
                    ps = ppool.tile([128, N], f32, tag="ps")
                    for t in range(9):
                        k, l = divmod(t, 3)
                        rv = rhs3[:, c0 + k : c0 + k + wc, l : l + DL]
                        nc.tensor.matmul(
                            ps.rearrange("m (c d) -> m c d", c=wc),
                            w_main_t[:, t * 128 : (t + 1) * 128],
                            rv,
                            start=(t == 0),
                            stop=(t == 8),
                        )
                    ot = opool.tile([128, N], f32, tag="ot")
                    nc.vector.tensor_scalar_add(ot[:], ps[:], bias_main_t[:])
                    nc.scalar.dma_start(out[bb, cc, :, :N], ot[:])

            # last block (b_out 28:30, M=64): run c-chunks as column-tiled
            # pairs — chunk 2p in PE columns 0:64, chunk 2p+1 in 64:128.
            # p=3 (the ragged pair with a split store) runs first so the
            # kernel's final store is a single clean block.
            rhs3 = rhs_tiles[NBB - 1].rearrange("p (c d) -> p c d", c=C)
            for p in (3, 0, 1, 2):
                wc_b = 8 if p < 3 else 6
                N_b = wc_b * DL
                ps = ppool.tile([128, 8 * DL], f32, tag="ps")
                ot = opool.tile([128, 8 * DL], f32, tag="ot")
                # the final pair (p=2) computes in two column phases so the
                # left phase's bias-add + store overlap the right phase's
                # matmuls, shortening the kernel's tail chain
                phases = [(0, 5), (5, 3)] if p == 2 else [(0, 8)]
                for pc0, pwc in phases:
                    for t in range(9):
                        k, l = divmod(t, 3)
                        for half, wch in enumerate([8, wc_b]):
                            w0 = min(pc0, wch)
                            w1 = min(pc0 + pwc, wch)
                            if w1 <= w0:
                                continue
                            c0 = 16 * p + 8 * half + w0
                            rv = rhs3[:, c0 + k : c0 + k + (w1 - w0), l : l + DL]
                            nc.tensor.matmul(
                                ps[
                                    64 * half : 64 * half + 64, w0 * DL : w1 * DL
                                ].rearrange("m (c d) -> m c d", c=w1 - w0),
                                w_last_t[:, t * 64 : (t + 1) * 64],
                                rv,
                                start=(t == 0),
                                stop=(t == 8),
                                tile_position=(0, 64 * half),
                            )
                    if p == 2:
                        lo, hi = pc0 * DL, (pc0 + pwc) * DL
                        nc.vector.tensor_scalar_add(
                            ot[:, lo:hi], ps[:, lo:hi], bias_pair_t[:]
                        )
                        if pc0 == 0:
                            nc.scalar.dma_start(
                                out[NBB - 1, 2 * p, :, lo:hi], ot[:, lo:hi]
                            )
                        else:
                            # final store: split by PARTITIONS (descriptors
                            # are per partition-row, so this halves the
                            # descriptor count per ring-set)
                            nc.sync.dma_start(
                                out[NBB - 1, 2 * p, :64, lo:hi], ot[:64, lo:hi]
                            )
                            nc.gpsimd.dma_start(
                                out[NBB - 1, 2 * p, 64:, lo:hi], ot[64:, lo:hi]
                            )
                if p == 1:
                    # second-to-last store on the (idle) gpsimd rings so it
                    # doesn't queue ahead of the final stores
                    nc.vector.tensor_scalar_add(ot[:], ps[:], bias_pair_t[:])
                    nc.gpsimd.dma_start(out[NBB - 1, 2 * p, :, :], ot[:])
                elif p == 0:
                    nc.vector.tensor_scalar_add(ot[:], ps[:], bias_pair_t[:])
                    nc.scalar.dma_start(out[NBB - 1, 2 * p, :, :], ot[:])
                elif p == 3:
                    nc.vector.tensor_scalar_add(
                        ot[:, :N_b], ps[:, :N_b], bias_pair_t[:]
                    )
                    nc.vector.tensor_scalar_add(
                        ot[:64, N_b : 8 * DL],
                        ps[:64, N_b : 8 * DL],
                        bias_pair_t[:64],
                    )
                    nc.scalar.dma_start(out[NBB - 1, 2 * p, :, :N_b], ot[:, :N_b])
                    nc.scalar.dma_start(
                        out[NBB - 1, 2 * p, :64, N_b : 8 * DL],
                        ot[:64, N_b : 8 * DL],
                    )
    nc.compile()
    return nc


def kernel(x: np.ndarray, weight: np.ndarray, bias: np.ndarray) -> np.ndarray:
    from concourse.bass_utils import run_bass_kernel_spmd

    if "nc" not in _CACHE:
        _CACHE["nc"] = _build_program()
    nc = _CACHE["nc"]

    w_main, w_last, bias_main, bias_pair = _build_weights(weight, bias)
    x_bf = x.astype(BF16)
    w_main = w_main.astype(BF16)
    w_last = w_last.astype(BF16)

    in_maps = []
    for core in range(8):
        b, q = divmod(core, 4)
        a0 = A0[q]
        slab = x_bf[b, :, a0 : a0 + SA]  # (ci, a, b2, c, d)
        xp = np.ascontiguousarray(slab.transpose(2, 0, 1, 3, 4)).reshape(
            B2 * CI * SA, C * D
        )
        in_maps.append(
            {
                "xp": xp,
                "w_main": w_main,
                "w_last": w_last,
                "bias_main": bias_main,
                "bias_pair": bias_pair,
            }
        )

    res = run_bass_kernel_spmd(nc, in_maps, core_ids=list(range(8)))
    _CACHE["last_result"] = res

    out = np.empty((B, CO, AO, BO, CL, DL), np.float32)
    for core in range(8):
        b, q = divmod(core, 4)
        slab = _unscramble(res.results[core]["out_blocks"])  # (4, 8, 30, 62, 62)
        if q < 3:
            out[b, :, 8 * q : 8 * q + 8] = slab
        else:
            out[b, :, 24:30] = slab[:, 2:8]
    return out


def _unscramble(blocks: np.ndarray) -> np.ndarray:
    """[NBB, NCC, 128, 8*62] partition-major blocks -> (4, 8, 30, 62, 62) slab."""
    slab = np.empty((CO, 8, BO, CL, DL), np.float32)
    for bb in range(NBB - 1):
        for cc in range(NCC):
            wc = 8 if cc < NCC - 1 else 6
            n = wc * DL
            blk = blocks[bb, cc, :, :n].reshape(CO, 8, 4, wc, DL)
            slab[:, :, bb * 4 : bb * 4 + 4, cc * 8 : cc * 8 + wc, :] = blk
    # last b-block: pairs stored at [7, 2p]; top half = chunk 2p (wc=8),
    # bottom half = chunk 2p+1 (wc=8, or 6 for p=3)
    for p in range(4):
        blk = blocks[NBB - 1, 2 * p]
        top = blk[:64, : 8 * DL].reshape(CO, 8, 2, 8, DL)
        slab[:, :, 28:30, 16 * p : 16 * p + 8, :] = top
        wc_b = 8 if p < 3 else 6
        bot = blk[64:128, : wc_b * DL].reshape(CO, 8, 2, wc_b, DL)
        slab[:, :, 28:30, 16 * p + 8 : 16 * p + 8 + wc_b, :] = bot
    return slab
